# revision 26
# baseline (speedup 1.0000x reference)
"""Multi-head attention (B=16, L=S=1024, D=P=512, H=8) on 8 TRN2 NeuronCores.

Strategy: pure data parallelism over the batch — each core computes the full
attention block for 2 batch elements.  Activations are fed to the device
pre-transposed ([D, L] instead of [L, D]) so every GEMM contracts over the
partition dimension with no on-chip transposes:

  per batch element b (all on one core):
    QT[P,L] = Wq.T @ qT      (lhsT = Wq tile,   rhs = qT)   + bq (per-partition)
    KT[P,S] = Wk.T @ kT      (lhsT = Wk tile,   rhs = kT)   + bk (per-partition)
    V [S,P] = vT.T @ Wv      (lhsT = vT tile,   rhs = Wv)   + bv (free-dim row)
    per head h (E=64), software-pipelined one (h, L-chunk) ahead:
      expT[S,L] = exp(scale * K_h @ Q_h^T)     (scores transposed so the A@V
      OT_h[E,L] = V65_h.T @ expT                matmul needs no transposes; V
                                                carries a ones column per head,
                                                so psum row 64 = softmax sums)
      OT_h *= 1/sums  (fast-approx recip + gpsimd partition_broadcast)
    out[L,D] = OT.T-contraction with Wo + bo

All matmuls run in float32r (TRN2's fast fp32 mode: 1 cycle/row at free-dim
>= 256, and per neuronxcc more precise than the 4-cycle fp32 path).  Softmax
skips the max-subtraction: scaled scores are ~N(0, 0.2), so exp() is safe and
matches the reference within fp32 rounding (measured ~1.4e-4 rel err on HW).
"""

import numpy as np

B, L, S, D, P, H, E = 16, 1024, 1024, 512, 512, 8, 64
NCORES = 8
BPC = B // NCORES  # batch elements per core
SCALE = 1.0 / float(np.sqrt(E))

_CACHE = {}
LAST_RESULTS = None  # stashed BassKernelResults for test harness introspection
MM_DTYPE = "f32r"  # "f32r" (precise) | "bf16" (fast) | "hybrid" (bf16 A@V only)


def _build():
    """Build (once) the Bass program executed identically on all 8 cores."""
    if "nc" in _CACHE:
        return _CACHE["nc"]

    from contextlib import ExitStack

    import concourse.bass as bass
    import concourse.mybir as mybir
    import concourse.tile as tile
    from concourse import bacc

    f32 = mybir.dt.float32
    f32r = mybir.dt.bfloat16 if MM_DTYPE == "bf16" else mybir.dt.float32r
    fin = mybir.dt.bfloat16 if MM_DTYPE == "bf16" else f32
    fav = mybir.dt.bfloat16 if MM_DTYPE in ("bf16", "hybrid") else f32r
    AF = mybir.ActivationFunctionType

    nc = bacc.Bacc("TRN2", target_bir_lowering=False, debug=False)

    qT = nc.dram_tensor("qT", [BPC, D, L], fin, kind="ExternalInput").ap()
    kT = nc.dram_tensor("kT", [BPC, D, S], fin, kind="ExternalInput").ap()
    vT = nc.dram_tensor("vT", [BPC, D, S], fin, kind="ExternalInput").ap()
    Wq = nc.dram_tensor("Wq", [D, P], fin, kind="ExternalInput").ap()
    Wk = nc.dram_tensor("Wk", [D, P], fin, kind="ExternalInput").ap()
    Wv = nc.dram_tensor("Wv", [D, P], fin, kind="ExternalInput").ap()
    Wo = nc.dram_tensor("Wo", [P, D], fin, kind="ExternalInput").ap()
    bq_col = nc.dram_tensor("bq_col", [128, 4], f32, kind="ExternalInput").ap()
    bk_col = nc.dram_tensor("bk_col", [128, 4], f32, kind="ExternalInput").ap()
    bv_row = nc.dram_tensor("bv_row", [P], f32, kind="ExternalInput").ap()
    bo_row = nc.dram_tensor("bo_row", [D], f32, kind="ExternalInput").ap()
    ones_in = nc.dram_tensor("ones_in", [128, 128], fin, kind="ExternalInput").ap()
    out = nc.dram_tensor("out", [BPC, L, D], f32, kind="ExternalOutput").ap()

    def bcast_ap(src):
        # [N] DRAM vector -> [128, N] partition-broadcast access pattern
        return bass.AP(tensor=src.tensor, offset=src.offset, ap=[[0, 128]] + src.ap)

    with tile.TileContext(nc) as tc, ExitStack() as ctx:
        consts = ctx.enter_context(tc.tile_pool(name="consts", bufs=1))
        xT_pool = ctx.enter_context(tc.tile_pool(name="xT", bufs=2))
        acts = ctx.enter_context(tc.tile_pool(name="acts", bufs=1))
        exp_pool = ctx.enter_context(tc.tile_pool(name="exp", bufs=3))
        small = ctx.enter_context(tc.tile_pool(name="small", bufs=2))
        out_pool = ctx.enter_context(tc.tile_pool(name="outp", bufs=2))
        psum = ctx.enter_context(tc.tile_pool(name="psum", bufs=2, space="PSUM"))

        # ---- constants: weights [128, dtile, N] with contraction dim on partitions.
        # DMA issue order is interleaved with the first batch's activation loads
        # below so the first projection matmul isn't queued behind 4MB of weights.
        Wq_sb = consts.tile([128, 4, P], f32r, tag="Wq")
        Wk_sb = consts.tile([128, 4, P], f32r, tag="Wk")
        Wv_sb = consts.tile([128, 4, P], f32r, tag="Wv")
        Wo_sb = consts.tile([128, 4, D], f32r, tag="Wo")
        bq_sb = consts.tile([128, 4], f32, tag="bq")
        bk_sb = consts.tile([128, 4], f32, tag="bk")
        bv_sb = consts.tile([128, P], f32, tag="bv")
        bo_sb = consts.tile([128, D], f32, tag="bo")
        ones_sb = consts.tile([128, 128], f32r, tag="ones")

        def load_xT(src, b, name, split_first=False):
            # one tile per dtile (released independently -> 5 ring slots cover
            # a tensor plus the next one's prefetch); split_first halves the
            # dt=0 load so the very first matmul starts sooner.
            view = src[b].rearrange("(t p) l -> p t l", p=128).bitcast(f32r)
            ts = []
            for dt in range(4):
                t = xT_pool.tile([128, L], f32r, tag="xT", name=name, bufs=5)
                if dt == 0 and split_first:
                    nc.sync.dma_start(out=t[:, 0:512], in_=view[:, 0, 0:512])
                    nc.sync.dma_start(out=t[:, 512:L], in_=view[:, 0, 512:L])
                else:
                    nc.sync.dma_start(out=t, in_=view[:, dt, :])
                ts.append(t)
            return ts

        def load_w(W_sb, Wsrc):
            view = Wsrc.rearrange("(t p) n -> p t n", p=128).bitcast(f32r)
            for dt in range(4):
                nc.sync.dma_start(out=W_sb[:, dt, :], in_=view[:, dt, :])

        # first matmul needs only Wq[dt0] + qT[dt0]: issue those two first
        Wq_view = Wq.rearrange("(t p) n -> p t n", p=128).bitcast(f32r)
        nc.sync.dma_start(out=Wq_sb[:, 0, :], in_=Wq_view[:, 0, :])
        first = {"qT_sb": load_xT(qT, 0, "qT_sb", split_first=True)}
        for dt in range(1, 4):
            nc.sync.dma_start(out=Wq_sb[:, dt, :], in_=Wq_view[:, dt, :])
        nc.sync.dma_start(out=bq_sb, in_=bq_col)
        load_w(Wk_sb, Wk)
        nc.sync.dma_start(out=bk_sb, in_=bk_col)
        first["kT_sb"] = load_xT(kT, 0, "kT_sb")
        load_w(Wv_sb, Wv)
        nc.gpsimd.dma_start(out=bv_sb, in_=bcast_ap(bv_row))
        first["vT_sb"] = load_xT(vT, 0, "vT_sb")
        load_w(Wo_sb, Wo)
        nc.gpsimd.dma_start(out=bo_sb, in_=bcast_ap(bo_row))
        nc.sync.dma_start(out=ones_sb, in_=ones_in.bitcast(f32r))

        for b in range(BPC):
            if b == 0:
                qT_sb, kT_sb, vT_sb = first["qT_sb"], first["kT_sb"], first["vT_sb"]
            else:
                qT_sb = load_xT(qT, b, "qT_sb")
                kT_sb = load_xT(kT, b, "kT_sb")
                vT_sb = load_xT(vT, b, "vT_sb")

            QT_sb = acts.tile([128, 4, L], f32r, tag="QT")  # [P-part, ptile, L]
            KT_sb = acts.tile([128, 4, S], f32r, tag="KT")
            # V in 65-wide head blocks: cols h*65..h*65+63 = head h of V,
            # col h*65+64 = 1.0 -- so the OT matmul's stationary [128,65]
            # emits the softmax denominator as psum row 64 for free.
            V_sb = acts.tile([128, 8, 8 * 65], fav, tag="V")  # [S-part, stile, 520]
            Vv = V_sb.rearrange("p s (h e) -> p s h e", e=65)
            OT_sb = acts.tile([128, 4, L], f32r, tag="OT")  # [P-part, ptile, L]

            nc.vector.tensor_copy(
                Vv[:, :, :, 64], ones_sb[:, 0:64].rearrange("p (s h) -> p s h", s=8)
            )

            # ---- QT / KT projections: psum[p, l] = sum_d W[d, p] * xT[d, l]
            for W_sb, b_sb, X_sb, Y_sb in (
                (Wq_sb, bq_sb, qT_sb, QT_sb),
                (Wk_sb, bk_sb, kT_sb, KT_sb),
            ):
                for pt in range(4):
                    for lc in range(2):
                        ps = psum.tile([128, 512], f32, tag="proj")
                        for dt in range(4):
                            nc.tensor.matmul(
                                ps,
                                W_sb[:, dt, pt * 128:(pt + 1) * 128],
                                X_sb[dt][:, lc * 512:(lc + 1) * 512],
                                start=(dt == 0),
                                stop=(dt == 3),
                            )
                        nc.vector.tensor_scalar_add(
                            Y_sb[:, pt, lc * 512:(lc + 1) * 512], ps, b_sb[:, pt:pt + 1]
                        )

            # ---- V projection: psum[s, p] = sum_d vT[d, s] * Wv[d, p]
            for st in range(8):
                ps = psum.tile([128, 512], f32, tag="proj")
                for dt in range(4):
                    nc.tensor.matmul(
                        ps,
                        vT_sb[dt][:, st * 128:(st + 1) * 128],
                        Wv_sb[:, dt, :],
                        start=(dt == 0),
                        stop=(dt == 3),
                    )
                nc.vector.tensor_add(
                    Vv[:, st, :, 0:64],
                    ps.rearrange("p (h e) -> p h e", e=64),
                    bv_sb.rearrange("p (h e) -> p h e", e=64),
                )

            # ---- attention, head-PAIR interleaved + software-pipelined.
            # Heads 2hp (rows 0-63) and 2hp+1 (rows 64-127) of QT/KT tile hp are
            # processed together, alternating their scores matmuls MM-by-MM: the
            # two heads use disjoint PE row groups, so each LDWEIGHTS overlaps
            # the other head's streaming matmul.  One psum tile per S-tile holds
            # both heads' scores [h0 | h1]; one exp op drains both.
            def emit_pair_scores_exp(hp, lc):
                lsl = slice(lc * 512, (lc + 1) * 512)
                expT_c = exp_pool.tile([128, 8, 1024], fav, tag="expT", name="expT_c", bufs=2)
                for st in range(8):
                    ps_s = psum.tile([128, 1024], f32, tag="scores", name="ps_s")
                    for hh in range(2):
                        po = hh * 64
                        nc.tensor.matmul(
                            ps_s[:, hh * 512:(hh + 1) * 512],
                            KT_sb[po:po + 64, hp, st * 128:(st + 1) * 128],
                            QT_sb[po:po + 64, hp, lsl],
                            start=True,
                            stop=True,
                        )
                    nc.scalar.activation(
                        out=expT_c[:, st, :], in_=ps_s, func=AF.Exp, scale=SCALE
                    )
                return expT_c

            def emit_pair_ot_norm(hp, lc, expT_c):
                lsl = slice(lc * 512, (lc + 1) * 512)
                for hh in range(2):
                    h = 2 * hp + hh
                    po_h = hh * 64
                    ps_o = psum.tile([65, 512], f32, tag="ot", name="ps_o")
                    for st in range(8):
                        nc.tensor.matmul(
                            ps_o,
                            V_sb[:, st, h * 65:(h + 1) * 65],
                            expT_c[:, st, hh * 512:(hh + 1) * 512],
                            start=(st == 0),
                            stop=(st == 7),
                        )
                    # custom-DVE ops misread PSUM partition offsets (HW bug):
                    # stage the sums row through SBUF before the fast recip.
                    sums_sb = small.tile([1, 512], f32, tag="sums", name="sums_sb", bufs=1)
                    nc.vector.tensor_copy(sums_sb, ps_o[64:65, :])
                    recip_sb = small.tile([1, 512], f32, tag="recip", name="recip_sb")
                    nc.vector.reciprocal_approx_fast(out=recip_sb, in_=sums_sb)
                    rep_sb = small.tile([64, 512], f32, tag="rep", name="rep_sb")
                    nc.gpsimd.partition_broadcast(rep_sb, recip_sb, channels=64)
                    nc.vector.tensor_mul(
                        OT_sb[po_h:po_h + 64, hp, lsl], ps_o[0:64, :], rep_sb
                    )

            pending = None
            for hp in range(4):
                for lc in range(2):
                    expT_c = emit_pair_scores_exp(hp, lc)
                    if pending is not None:
                        emit_pair_ot_norm(*pending)
                    pending = (hp, lc, expT_c)
            emit_pair_ot_norm(*pending)

            # ---- out projection: psum[l, d] = sum_p OT[p, l] * Wo[p, d]
            for lt in range(8):
                ps = psum.tile([128, 512], f32, tag="proj")
                for pt in range(4):
                    nc.tensor.matmul(
                        ps,
                        OT_sb[:, pt, lt * 128:(lt + 1) * 128],
                        Wo_sb[:, pt, :],
                        start=(pt == 0),
                        stop=(pt == 3),
                    )
                o_sb = out_pool.tile([128, 512], f32, tag="osb")
                nc.vector.tensor_add(o_sb, ps, bo_sb)
                nc.sync.dma_start(out=out[b, lt * 128:(lt + 1) * 128, :], in_=o_sb)

    nc.compile()
    _CACHE["nc"] = nc
    return nc


def _in_maps(inputs):
    if MM_DTYPE == "bf16":
        import ml_dtypes
        mm_np = ml_dtypes.bfloat16
    else:
        mm_np = np.float32
    g = lambda a: np.ascontiguousarray(np.asarray(a, dtype=np.float32).astype(mm_np))
    f = lambda a: np.ascontiguousarray(np.asarray(a, dtype=np.float32))
    queries, keys, values = f(inputs["queries"]), f(inputs["keys"]), f(inputs["values"])
    Wq, Wk, Wv, Wo = g(inputs["Wq"]), g(inputs["Wk"]), g(inputs["Wv"]), g(inputs["Wo"])
    bq, bk, bv, bo = f(inputs["bq"]), f(inputs["bk"]), f(inputs["bv"]), f(inputs["bo"])
    shared = {
        "Wq": Wq, "Wk": Wk, "Wv": Wv, "Wo": Wo,
        "bq_col": np.ascontiguousarray(bq.reshape(4, 128).T),
        "bk_col": np.ascontiguousarray(bk.reshape(4, 128).T),
        "bv_row": bv, "bo_row": bo,
        "ones_in": np.ones((128, 128), mm_np),
    }
    maps = []
    for c in range(NCORES):
        sl = slice(BPC * c, BPC * (c + 1))
        maps.append({
            "qT": np.ascontiguousarray(queries[sl].transpose(0, 2, 1).astype(mm_np)),
            "kT": np.ascontiguousarray(keys[sl].transpose(0, 2, 1).astype(mm_np)),
            "vT": np.ascontiguousarray(values[sl].transpose(0, 2, 1).astype(mm_np)),
            **shared,
        })
    return maps


def kernel(**inputs) -> np.ndarray:
    global LAST_RESULTS
    from concourse import bass_utils

    nc = _build()
    maps = _in_maps(inputs)
    res = bass_utils.run_bass_kernel_spmd(nc, maps, core_ids=list(range(NCORES)))
    LAST_RESULTS = res
    return np.concatenate([res.results[c]["out"] for c in range(NCORES)], axis=0)
